# revision 1
# baseline (speedup 1.0000x reference)
import math
import numpy as np

# nn_AttentionLSTM: N=256, T=128, D=1024, H=1024, DA=7 (L=49)
# Sharding: data-parallel over N across 8 NeuronCores (32 samples/core).
# All recurrent state (h, c) and attention are per-sample, so no
# cross-core communication is needed; weights are replicated.

N, T, D, H, L = 256, 128, 1024, 1024, 49
NCORES = 8


def _build(jax, jnp):
    from functools import partial

    devs = jax.devices()[:NCORES]
    scale = 1.0 / math.sqrt(H)

    @partial(jax.pmap, devices=devs)
    def run(x_sh, A_sh, Wx_, Wh_, Wattn_, b_):
        h0 = A_sh.mean(axis=-1)

        def step(carry, xt):
            prev_h, prev_c = carry
            scores = jnp.einsum(
                'nhl,nh->nl', A_sh, prev_h,
                precision=jax.lax.Precision.HIGHEST) * scale
            w = jax.nn.softmax(scores, axis=1)
            attn = jnp.einsum(
                'nl,nhl->nh', w, A_sh,
                precision=jax.lax.Precision.HIGHEST)
            a = (jnp.dot(xt, Wx_, precision=jax.lax.Precision.HIGHEST)
                 + jnp.dot(prev_h, Wh_, precision=jax.lax.Precision.HIGHEST)
                 + jnp.dot(attn, Wattn_, precision=jax.lax.Precision.HIGHEST)
                 + b_)
            i = jax.nn.sigmoid(a[:, :H])
            f = jax.nn.sigmoid(a[:, H:2 * H])
            o = jax.nn.sigmoid(a[:, 2 * H:3 * H])
            g = jnp.tanh(a[:, 3 * H:])
            next_c = f * prev_c + i * g
            next_h = o * jnp.tanh(next_c)
            return (next_h, next_c), next_h

        _, hs = jax.lax.scan(step, (h0, h0), jnp.swapaxes(x_sh, 0, 1))
        return jnp.swapaxes(hs, 0, 1)

    return run


_cached = {}


def kernel(x, A, Wx, Wh, Wattn, b):
    import jax
    import jax.numpy as jnp

    if 'run' not in _cached:
        _cached['run'] = _build(jax, jnp)
    run = _cached['run']

    n = x.shape[0]
    ns = n // NCORES
    x_sh = np.ascontiguousarray(np.asarray(x).reshape(NCORES, ns, T, D))
    A_flat = np.asarray(A).reshape(n, H, L)
    A_sh = np.ascontiguousarray(A_flat.reshape(NCORES, ns, H, L))
    rep = lambda a: np.broadcast_to(np.asarray(a), (NCORES,) + np.asarray(a).shape)

    out = run(x_sh, A_sh, rep(Wx), rep(Wh), rep(Wattn), rep(b))
    out = np.asarray(out)
    return out.reshape(n, T, H)
